# revision 26
# baseline (speedup 1.0000x reference)
"""Trainium2 Bass kernel for nn_PolicyValueNet (GNN policy/value net).

Strategy: data-parallel over graphs (256 graphs per core x 8 cores).
Per core, graphs are packed in PAIRS (2 x 64 nodes = 128 = partition width).
The per-layer mean-aggregation segment reduction is computed on-device as a
dense block-diagonal adjacency matmul:
    aggT[f, d] = sum_s h[s, f] * A_norm[d, s]   (per pair, PE matmul)
with h kept feature-major [128 f, nodes] in SBUF, transposed per-pair on the
PE (matmul with identity) to get the node-major operand.

Host-side prep is restricted to data layout: slicing/transposing inputs,
folding constant scale factors into weights, and converting the edge list
into the dense per-pair adjacency count matrices (np.bincount histogram).
"""

import sys
import os
from contextlib import ExitStack

import numpy as np
import ml_dtypes

for _p in ("/opt/trn_rl_repo", "/root/.axon_site/_ro/trn_rl_repo"):
    if os.path.isdir(_p) and _p not in sys.path:
        sys.path.insert(0, _p)

import concourse.bass as bass
import concourse.bacc as bacc
import concourse.tile as tile
from concourse import mybir
from concourse.bass_utils import run_bass_kernel_spmd

# ---- problem dims (hardcoded per spec) ----
B, NN, D, HD, L, H, S, FIN = 2048, 64, 128, 128, 4, 8, 8, 16
N = B * NN            # 131072 total nodes
E = B * NN * 8        # 1048576 total edges
A_ = S * (S - 1)      # 56 actions per head
NCORES = 8
BG = B // NCORES      # 256 graphs per core
NPC = BG * NN         # 16384 nodes per core
PAIRS = BG // 2       # 128 graph-pairs per core
GROUPS = 4            # pair groups per core (32 pairs = 4096 nodes each)
PPG = PAIRS // GROUPS  # 32 pairs per group
CH = 512              # node-column chunk (= 4 pairs)
F32 = mybir.dt.float32
F32R = mybir.dt.float32r
BF16 = mybir.dt.bfloat16


def _r(ap):
    return ap.bitcast(F32R)

_CACHE = {}


def _build_nc():
    """Build the per-core Bass program (same program for all 8 cores)."""
    nc = bacc.Bacc(None, target_bir_lowering=False, debug=False)

    # ---- external inputs ----
    xT = nc.dram_tensor("xT", [FIN, NPC], F32R, kind="ExternalInput")
    Ablk = nc.dram_tensor("Ablk", [128, PAIRS * 128], BF16, kind="ExternalInput")
    eye = nc.dram_tensor("eye", [128, 128], F32, kind="ExternalInput")
    eye_b = nc.dram_tensor("eye_b", [128, 128], BF16, kind="ExternalInput")
    Win = nc.dram_tensor("Win", [FIN, D], F32R, kind="ExternalInput")
    b_in = nc.dram_tensor("b_in", [D, 1], F32, kind="ExternalInput")
    Wself = nc.dram_tensor("Wself", [L, D, D], F32R, kind="ExternalInput")
    Wnbr = nc.dram_tensor("Wnbr", [L, D, D], F32R, kind="ExternalInput")
    benc = nc.dram_tensor("benc", [L, D, 1], F32, kind="ExternalInput")
    extWm = nc.dram_tensor("extWm", [H, D, HD], F32, kind="ExternalInput")
    extWx = nc.dram_tensor("extWx", [H, D, HD], F32, kind="ExternalInput")
    extb = nc.dram_tensor("extb", [H, 1, HD], F32, kind="ExternalInput")
    ones1 = nc.dram_tensor("ones1", [1, 128], F32, kind="ExternalInput")
    lngb = nc.dram_tensor("lngb", [128, H * HD], F32, kind="ExternalInput")
    lnbb = nc.dram_tensor("lnbb", [128, H * HD], F32, kind="ExternalInput")
    hubW = nc.dram_tensor("hubW", [HD, HD], F32R, kind="ExternalInput")
    hubb = nc.dram_tensor("hubb", [HD, 1], F32, kind="ExternalInput")
    pW1a = nc.dram_tensor("pW1a", [H, HD, HD], F32R, kind="ExternalInput")
    pW1b = nc.dram_tensor("pW1b", [H, HD, HD], F32R, kind="ExternalInput")
    pb1 = nc.dram_tensor("pb1", [H, HD, 1], F32, kind="ExternalInput")
    pW2 = nc.dram_tensor("pW2", [H, HD, A_], F32R, kind="ExternalInput")
    pb2 = nc.dram_tensor("pb2", [H, A_, 1], F32, kind="ExternalInput")
    vW1 = nc.dram_tensor("vW1", [D, D], F32, kind="ExternalInput")
    vb1 = nc.dram_tensor("vb1", [D, 1], F32, kind="ExternalInput")
    vW2 = nc.dram_tensor("vW2", [D, 1], F32, kind="ExternalInput")
    vb2 = nc.dram_tensor("vb2", [1, 1], F32, kind="ExternalInput")

    # ---- external outputs ----
    # logits per graph flattened [BG, H*A_]; value [1, BG] (+vb2/tanh on device)
    lg_out = nc.dram_tensor("logits", [BG, H * A_], F32, kind="ExternalOutput")
    v_out = nc.dram_tensor("v", [1, BG], F32, kind="ExternalOutput")

    AF = mybir.ActivationFunctionType
    OP = mybir.AluOpType
    AX = mybir.AxisListType

    with tile.TileContext(nc) as tc, ExitStack() as top:
        persist = top.enter_context(tc.tile_pool(name="persist", bufs=1))
        # persistent SBUF state
        Hbuf = persist.tile([128, NPC], F32)          # h feature-major
        eye_t = persist.tile([128, 128], F32)
        eyeb_t = persist.tile([128, 128], BF16)
        Hb16 = persist.tile([128, NPC], BF16)
        win_t = persist.tile([FIN, D], F32R)
        bin_t = persist.tile([D, 1], F32)
        ws_t = [persist.tile([D, D], F32R, tag=f"ws{l}", name=f"ws{l}") for l in range(L)]
        wn_t = [persist.tile([D, D], F32R, tag=f"wn{l}", name=f"wn{l}") for l in range(L)]
        be_t = [persist.tile([D, 1], F32, tag=f"be{l}", name=f"be{l}") for l in range(L)]

        nc.sync.dma_start(out=eye_t[:], in_=eye[:])
        nc.sync.dma_start(out=eyeb_t[:], in_=eye_b[:])
        nc.sync.dma_start(out=win_t[:], in_=Win[:])
        nc.sync.dma_start(out=bin_t[:], in_=b_in[:])
        for l in range(L):
            nc.sync.dma_start(out=ws_t[l][:], in_=Wself[l])
            nc.sync.dma_start(out=wn_t[l][:], in_=Wnbr[l])
            nc.sync.dma_start(out=be_t[l][:], in_=benc[l])

        xpool = top.enter_context(tc.tile_pool(name="xp", bufs=2))

        with ExitStack() as lay:
            apool = lay.enter_context(tc.tile_pool(name="apool", bufs=1))
            Abuf = apool.tile([128, PAIRS * 128], BF16)
            for q in range(8):
                cs = q * (PAIRS * 128 // 8)
                nc.sync.dma_start(out=Abuf[:, cs:cs + PAIRS * 128 // 8],
                                  in_=Ablk[:, cs:cs + PAIRS * 128 // 8])

            # ---- L layers of message passing ----
            psA = lay.enter_context(
                tc.tile_pool(name="psA", bufs=2, space=bass.MemorySpace.PSUM))
            psB = lay.enter_context(
                tc.tile_pool(name="psB", bufs=2, space=bass.MemorySpace.PSUM))
            psW = lay.enter_context(
                tc.tile_pool(name="psW", bufs=4, space=bass.MemorySpace.PSUM))
            hp_pool = lay.enter_context(tc.tile_pool(name="hp", bufs=4))
            at_pool = lay.enter_context(tc.tile_pool(name="at", bufs=9))

            # ---- input projection: H = relu(x @ Win + b_in), feature-major ----
            XS = 2048
            for q in range(NPC // XS):
                xst = xpool.tile([FIN, XS], F32R, tag="xst", name="xst")
                nc.sync.dma_start(out=xst[:], in_=xT[:, q * XS:(q + 1) * XS])
                for c in range(XS // CH):
                    col = q * XS + c * CH
                    ps = psW.tile([128, CH], F32, tag="psW", name="psWi")
                    nc.tensor.matmul(ps[:], _r(win_t[:]),
                                     _r(xst[:, c * CH:(c + 1) * CH]),
                                     start=True, stop=True)
                    nc.scalar.activation(_r(Hbuf[:, col:col + CH]), ps[:],
                                         AF.Relu, bias=bin_t[:, 0:1])
                    nc.vector.tensor_scalar(Hb16[:, col:col + CH], ps[:],
                                            bin_t[:, 0:1], 0.0,
                                            OP.add, OP.max)

            for l in range(L):
                for g in range(GROUPS):
                    at_tiles = []
                    for q in range(PPG // 4):      # 8 batches of 4 pairs
                        p0 = g * PPG + q * 4
                        # T1: transpose 4 pairs of H to node-major
                        ps_h = psA.tile([128, CH], F32, tag="psA", name="psA")
                        for j in range(4):
                            p = p0 + j
                            nc.tensor.matmul(
                                ps_h[:, j * 128:(j + 1) * 128],
                                Hb16[:, p * 128:(p + 1) * 128], eyeb_t[:],
                                start=True, stop=True)
                        hp = hp_pool.tile([128, CH], BF16, tag="hp", name="hp")
                        nc.vector.tensor_copy(hp[:], ps_h[:])
                        # M1: aggT[f, d] = sum_s h[s,f] * A_norm[d,s]
                        ps_a = psB.tile([128, CH], F32, tag="psB", name="psB")
                        for j in range(4):
                            p = p0 + j
                            nc.tensor.matmul(
                                ps_a[:, j * 128:(j + 1) * 128],
                                hp[:, j * 128:(j + 1) * 128],
                                Abuf[:, p * 128:(p + 1) * 128],
                                start=True, stop=True)
                        at = at_pool.tile([128, CH], F32, tag="at", name="at")
                        nc.vector.tensor_copy(_r(at[:]), ps_a[:])
                        at_tiles.append(at)
                    # W-stream for this group's 8 chunks of 512 nodes,
                    # in sub-batches of 4 so each weight load covers 4 matmuls
                    for sb in range(2):
                        pws = []
                        for qq in range(4):
                            q = sb * 4 + qq
                            c0 = (g * PPG + q * 4) * 128
                            ps_w = psW.tile([128, CH], F32, tag="psW",
                                            name="psW")
                            nc.tensor.matmul(ps_w[:], _r(ws_t[l][:]),
                                             _r(Hbuf[:, c0:c0 + CH]),
                                             start=True, stop=False)
                            pws.append(ps_w)
                        for qq in range(4):
                            q = sb * 4 + qq
                            c0 = (g * PPG + q * 4) * 128
                            ps_w = pws[qq]
                            nc.tensor.matmul(ps_w[:], _r(wn_t[l][:]),
                                             _r(at_tiles[q][:]),
                                             start=False, stop=True)
                            nc.scalar.activation(_r(Hbuf[:, c0:c0 + CH]),
                                                 ps_w[:], AF.Relu,
                                                 bias=be_t[l][:, 0:1])
                            nc.scalar.activation(Hb16[:, c0:c0 + CH], ps_w[:],
                                                 AF.Relu, bias=be_t[l][:, 0:1])

        # ======================= tail / heads =======================
        tp = top.enter_context(tc.tile_pool(name="tail", bufs=1))
        Pmean = tp.tile([128, BG * H], F32)   # sum over S=8 nodes (scale folded)
        Pmax = tp.tile([128, BG * H], F32)
        gmS = tp.tile([128, BG], F32)         # sum over 64 nodes (scale folded)

        for g in range(GROUPS):
            i0, i1 = g * 4096, (g + 1) * 4096
            o0, o1 = g * 512, (g + 1) * 512
            hsrc = Hbuf[:, i0:i1].rearrange("p (t e) -> p t e", e=8)
            nc.vector.tensor_reduce(Pmean[:, o0:o1], hsrc, AX.X, OP.add)
            nc.vector.tensor_reduce(Pmax[:, o0:o1], hsrc, AX.X, OP.max)
        nc.vector.tensor_reduce(
            gmS[:], Pmean[:].rearrange("p (t e) -> p t e", e=8), AX.X, OP.add)

        wpool = top.enter_context(tc.tile_pool(name="wts", bufs=1))
        ewm_t = [wpool.tile([D, HD], F32, tag=f"ewm{h}", name=f"ewm{h}") for h in range(H)]
        ewx_t = [wpool.tile([D, HD], F32, tag=f"ewx{h}", name=f"ewx{h}") for h in range(H)]
        eb_t = [wpool.tile([1, HD], F32, tag=f"eb{h}", name=f"eb{h}") for h in range(H)]
        ones_t = wpool.tile([1, 128], F32)
        lng_t = wpool.tile([128, H * HD], F32)
        lnb_t = wpool.tile([128, H * HD], F32)
        hubW_t = wpool.tile([HD, HD], F32R)
        hubb_t = wpool.tile([HD, 1], F32)
        w1a_t = [wpool.tile([HD, HD], F32R, tag=f"w1a{h}", name=f"w1a{h}") for h in range(H)]
        w1b_t = [wpool.tile([HD, HD], F32R, tag=f"w1b{h}", name=f"w1b{h}") for h in range(H)]
        pb1_t = [wpool.tile([HD, 1], F32, tag=f"pb1{h}", name=f"pb1{h}") for h in range(H)]
        w2_t = [wpool.tile([HD, A_], F32R, tag=f"w2{h}", name=f"w2{h}") for h in range(H)]
        pb2_t = [wpool.tile([A_, 1], F32, tag=f"pb2{h}", name=f"pb2{h}") for h in range(H)]
        vW1_t = wpool.tile([D, D], F32)
        vb1_t = wpool.tile([D, 1], F32)
        vW2_t = wpool.tile([D, 1], F32)
        vb2_t = wpool.tile([1, 1], F32)
        eps_t = wpool.tile([128, 1], F32)
        nc.vector.memset(eps_t[:], 1e-5)

        nc.sync.dma_start(out=ones_t[:], in_=ones1[:])
        nc.sync.dma_start(out=lng_t[:], in_=lngb[:])
        nc.sync.dma_start(out=lnb_t[:], in_=lnbb[:])
        nc.sync.dma_start(out=hubW_t[:], in_=hubW[:])
        nc.sync.dma_start(out=hubb_t[:], in_=hubb[:])
        nc.sync.dma_start(out=vW1_t[:], in_=vW1[:])
        nc.sync.dma_start(out=vb1_t[:], in_=vb1[:])
        nc.sync.dma_start(out=vW2_t[:], in_=vW2[:])
        nc.sync.dma_start(out=vb2_t[:], in_=vb2[:])
        for h in range(H):
            nc.sync.dma_start(out=ewm_t[h][:], in_=extWm[h])
            nc.sync.dma_start(out=ewx_t[h][:], in_=extWx[h])
            nc.sync.dma_start(out=eb_t[h][:], in_=extb[h])
            nc.sync.dma_start(out=w1a_t[h][:], in_=pW1a[h])
            nc.sync.dma_start(out=w1b_t[h][:], in_=pW1b[h])
            nc.sync.dma_start(out=pb1_t[h][:], in_=pb1[h])
            nc.sync.dma_start(out=w2_t[h][:], in_=pW2[h])
            nc.sync.dma_start(out=pb2_t[h][:], in_=pb2[h])

        psH = top.enter_context(
            tc.tile_pool(name="psH", bufs=2, space=bass.MemorySpace.PSUM))
        psS = top.enter_context(
            tc.tile_pool(name="psS", bufs=3, space=bass.MemorySpace.PSUM))
        st_pool = top.enter_context(tc.tile_pool(name="st", bufs=4))

        # head embeddings, graph-major: per g-chunk a [128 g, H*HD] tile
        HEg = [tp.tile([128, H * HD], F32, tag=f"heg{g0}", name=f"heg{g0}") for g0 in range(2)]
        HET = [tp.tile([HD, BG], F32, tag=f"het{h}", name=f"het{h}") for h in range(H)]

        Pm3 = Pmean[:].rearrange("p (g e) -> p e g", e=8)   # [128, h, g]
        Px3 = Pmax[:].rearrange("p (g e) -> p e g", e=8)

        for g0 in range(2):
            for h in range(H):
                gsl = slice(g0 * 128, (g0 + 1) * 128)
                ps_z = psH.tile([128, 128], F32, tag="ps128", name="psZ")
                nc.tensor.matmul(ps_z[:], Pm3[:, h, gsl], ewm_t[h][:],
                                 start=True, stop=False)
                nc.tensor.matmul(ps_z[:], Px3[:, h, gsl], ewx_t[h][:],
                                 start=False, stop=False)
                nc.tensor.matmul(ps_z[:], ones_t[0:1, :], eb_t[h][:],
                                 start=False, stop=True)
                # LayerNorm over hd (free dim) + affine + relu
                st6 = st_pool.tile([128, 6], F32, tag="st6", name="st6")
                nc.vector.bn_stats(st6[:], ps_z[:])
                mv = st_pool.tile([128, 2], F32, tag="mv", name="mv")
                nc.vector.bn_aggr(mv[:], st6[:])
                sd = st_pool.tile([128, 1], F32, tag="sd", name="sd")
                nc.scalar.activation(sd[:], mv[:, 1:2], AF.Sqrt, bias=eps_t[:, 0:1])
                rstd = st_pool.tile([128, 1], F32, tag="rstd", name="rstd")
                nc.vector.reciprocal(rstd[:], sd[:])
                zt = st_pool.tile([128, 128], F32, tag="zt", name="zt")
                nc.vector.tensor_scalar(zt[:], ps_z[:], mv[:, 0:1], rstd[:, 0:1],
                                        OP.subtract, OP.mult)
                hsl = slice(h * HD, (h + 1) * HD)
                zg = st_pool.tile([128, 128], F32, tag="zg", name="zg")
                nc.vector.tensor_mul(zg[:], zt[:], lng_t[:, hsl])
                za = st_pool.tile([128, 128], F32, tag="za", name="za")
                nc.vector.tensor_add(za[:], zg[:], lnb_t[:, hsl])
                nc.vector.tensor_scalar_max(HEg[g0][:, hsl], za[:], 0.0)
                # transpose to feature-major HET[h][:, g-chunk]
                ps_t = psH.tile([HD, 128], F32, tag="ps128", name="psT")
                nc.tensor.matmul(ps_t[:], HEg[g0][:, hsl], eye_t[:],
                                 start=True, stop=True)
                nc.scalar.copy(_r(HET[h][:, gsl]), ps_t[:])

        # hub: ctxT = relu(hubW'.T @ mean_h(head_emb) + hubb) (1/H folded into hubW)
        sumHE = tp.tile([HD, BG], F32)
        nc.vector.tensor_add(_r(sumHE[:]), HET[0][:], HET[1][:])
        for h in range(2, H):
            nc.vector.tensor_add(_r(sumHE[:]), sumHE[:], HET[h][:])
        ps_c = psH.tile([HD, BG], F32, tag="ps256", name="psC")
        nc.tensor.matmul(ps_c[:], _r(hubW_t[:]), _r(sumHE[:]), start=True, stop=True)
        ctxT = tp.tile([HD, BG], F32)
        nc.scalar.activation(_r(ctxT[:]), ps_c[:], AF.Relu, bias=hubb_t[:, 0:1])

        # policy heads
        Lout = [tp.tile([128, H * A_], F32, tag=f"lo{g0}", name=f"lo{g0}") for g0 in range(2)]
        for h in range(H):
            ps_hh = psH.tile([HD, BG], F32, tag="ps256", name="psHH")
            nc.tensor.matmul(ps_hh[:], _r(w1a_t[h][:]), _r(HET[h][:]),
                             start=True, stop=False)
            nc.tensor.matmul(ps_hh[:], _r(w1b_t[h][:]), _r(ctxT[:]),
                             start=False, stop=True)
            hhT = st_pool.tile([HD, BG], F32, tag="hhT", name="hhT")
            nc.scalar.activation(_r(hhT[:]), ps_hh[:], AF.Relu, bias=pb1_t[h][:, 0:1])
            ps_l = psS.tile([A_, BG], F32, tag="psS", name="psL")
            nc.tensor.matmul(ps_l[:], _r(w2_t[h][:]), _r(hhT[:]), start=True, stop=True)
            lgT = st_pool.tile([A_, BG], F32, tag="lgT", name="lgT")
            nc.scalar.activation(lgT[:], ps_l[:], AF.Copy, bias=0.0)
            nc.vector.tensor_scalar_add(lgT[:], lgT[:], pb2_t[h][:, 0:1])
            # transpose [A_, 128] chunks -> [128 g, A_]
            for g0 in range(2):
                ps_o = psS.tile([128, A_], F32, tag="psS", name="psO")
                nc.tensor.matmul(ps_o[:], lgT[:, g0 * 128:(g0 + 1) * 128],
                                 eye_t[0:A_, 0:A_], start=True, stop=True)
                nc.vector.tensor_copy(Lout[g0][:, h * A_:(h + 1) * A_], ps_o[:])

        for g0 in range(2):
            nc.sync.dma_start(out=lg_out[g0 * 128:(g0 + 1) * 128, :],
                              in_=Lout[g0][:])

        # value head (1/64 folded into vW1)
        ps_v = psH.tile([D, BG], F32, tag="ps256", name="psV")
        nc.tensor.matmul(ps_v[:], vW1_t[:], gmS[:], start=True, stop=True)
        vt = tp.tile([D, BG], F32)
        nc.scalar.activation(vt[:], ps_v[:], AF.Relu, bias=vb1_t[:, 0:1])
        ps_v2 = psS.tile([1, BG], F32, tag="psS", name="psV2")
        nc.tensor.matmul(ps_v2[:], vW2_t[:], vt[:], start=True, stop=True)
        vout = tp.tile([1, BG], F32)
        # vb2 added host-side? No: reference vb2 is zeros-initialized input;
        # fold via bias: tanh(in + vb2) with vb2 scalar folded at host into...
        nc.scalar.activation(vout[:], ps_v2[:], AF.Tanh, bias=vb2_t[0:1, 0:1])
        nc.sync.dma_start(out=v_out[:], in_=vout[:])

    nc.compile()
    return nc


def _host_prep(inputs):
    """Host-side layout prep; returns per-core in_maps."""
    x = np.ascontiguousarray(np.asarray(inputs["x"], dtype=np.float32))
    ei = np.asarray(inputs["edge_index"])
    W_in = np.asarray(inputs["W_in"], np.float32)
    b_in = np.asarray(inputs["b_in"], np.float32)
    W_self = np.asarray(inputs["W_self"], np.float32)
    W_nbr = np.asarray(inputs["W_nbr"], np.float32)
    b_enc = np.asarray(inputs["b_enc"], np.float32)
    ext_W = np.asarray(inputs["ext_W"], np.float32)
    ext_b = np.asarray(inputs["ext_b"], np.float32)
    ln_g = np.asarray(inputs["ln_g"], np.float32)
    ln_b = np.asarray(inputs["ln_b"], np.float32)
    hub_W = np.asarray(inputs["hub_W"], np.float32)
    hub_b = np.asarray(inputs["hub_b"], np.float32)
    pol_W1 = np.asarray(inputs["pol_W1"], np.float32)
    pol_b1 = np.asarray(inputs["pol_b1"], np.float32)
    pol_W2 = np.asarray(inputs["pol_W2"], np.float32)
    pol_b2 = np.asarray(inputs["pol_b2"], np.float32)
    val_W1 = np.asarray(inputs["val_W1"], np.float32)
    val_b1 = np.asarray(inputs["val_b1"], np.float32)
    val_W2 = np.asarray(inputs["val_W2"], np.float32)
    val_b2 = np.asarray(inputs["val_b2"], np.float32)

    src = ei[0].astype(np.int64)
    dst = ei[1].astype(np.int64)
    # dense per-pair adjacency histogram: A[pair, d_local, s_local] = edge count
    pair = dst >> 7
    idx = (pair << 14) | ((dst & 127) << 7) | (src & 127)
    counts = np.bincount(idx, minlength=(B // 2) * 128 * 128)
    counts = counts.reshape(B // 2, 128, 128).astype(np.float32)
    deg = counts.sum(axis=2)                      # segment_sum(ones, dst)
    A_norm = counts / np.maximum(deg, 1.0)[:, :, None]
    A_normT = np.ascontiguousarray(A_norm.transpose(0, 2, 1))  # [pair, s, d]

    # weight prep (shared across cores)
    shared = dict(
        eye=np.eye(128, dtype=np.float32),
        eye_b=np.eye(128, dtype=np.float32).astype(ml_dtypes.bfloat16),
        Win=W_in,
        b_in=b_in.reshape(D, 1),
        Wself=W_self,
        Wnbr=W_nbr,
        benc=b_enc.reshape(L, D, 1),
        extWm=np.ascontiguousarray(ext_W[:, :D, :]) / np.float32(S),
        extWx=np.ascontiguousarray(ext_W[:, D:, :]),
        extb=ext_b.reshape(H, 1, HD),
        ones1=np.ones((1, 128), np.float32),
        lngb=np.ascontiguousarray(
            np.broadcast_to(ln_g.reshape(1, H * HD), (128, H * HD))),
        lnbb=np.ascontiguousarray(
            np.broadcast_to(ln_b.reshape(1, H * HD), (128, H * HD))),
        hubW=hub_W / np.float32(H),
        hubb=hub_b.reshape(HD, 1),
        pW1a=np.ascontiguousarray(pol_W1[:, :HD, :]),
        pW1b=np.ascontiguousarray(pol_W1[:, HD:, :]),
        pb1=pol_b1.reshape(H, HD, 1),
        pW2=pol_W2,
        pb2=pol_b2.reshape(H, A_, 1),
        vW1=val_W1 / np.float32(NN),
        vb1=val_b1.reshape(D, 1),
        vW2=val_W2.reshape(D, 1),
        vb2=val_b2.reshape(1, 1),
    )

    in_maps = []
    for c in range(NCORES):
        n0 = c * NPC
        p0 = c * PAIRS
        m = dict(shared)
        m["xT"] = np.ascontiguousarray(x[n0:n0 + NPC].T)
        m["Ablk"] = np.ascontiguousarray(
            A_normT[p0:p0 + PAIRS].transpose(1, 0, 2).reshape(
                128, PAIRS * 128)).astype(ml_dtypes.bfloat16)
        in_maps.append(m)
    return in_maps


def get_nc():
    if "nc" not in _CACHE:
        _CACHE["nc"] = _build_nc()
    return _CACHE["nc"]


def run(inputs, trace=False):
    nc = get_nc()
    in_maps = _host_prep(inputs)
    res = run_bass_kernel_spmd(nc, in_maps, core_ids=list(range(NCORES)),
                               trace=trace)
    logits = np.concatenate(
        [r["logits"].reshape(BG, H, A_) for r in res.results], axis=0)
    v = np.concatenate(
        [r["v"].reshape(BG, 1) for r in res.results], axis=0)
    return (logits, v), res


def kernel(**inputs):
    (logits, v), _ = run(inputs, trace=False)
    return logits, v


# revision 27
# speedup vs baseline: 1.0434x; 1.0434x over previous
"""Trainium2 Bass kernel for nn_PolicyValueNet (GNN policy/value net).

Strategy: data-parallel over graphs (256 graphs per core x 8 cores).
Per core, graphs are packed in PAIRS (2 x 64 nodes = 128 = partition width).
The per-layer mean-aggregation segment reduction is computed on-device as a
dense block-diagonal adjacency matmul:
    aggT[f, d] = sum_s h[s, f] * A_norm[d, s]   (per pair, PE matmul)
with h kept feature-major [128 f, nodes] in SBUF, transposed per-pair on the
PE (matmul with identity) to get the node-major operand.

Host-side prep is restricted to data layout: slicing/transposing inputs,
folding constant scale factors into weights, and converting the edge list
into the dense per-pair adjacency count matrices (np.bincount histogram).
"""

import sys
import os
from contextlib import ExitStack

import numpy as np
import ml_dtypes

for _p in ("/opt/trn_rl_repo", "/root/.axon_site/_ro/trn_rl_repo"):
    if os.path.isdir(_p) and _p not in sys.path:
        sys.path.insert(0, _p)

import concourse.bass as bass
import concourse.bacc as bacc
import concourse.tile as tile
from concourse import mybir
from concourse.bass_utils import run_bass_kernel_spmd

# ---- problem dims (hardcoded per spec) ----
B, NN, D, HD, L, H, S, FIN = 2048, 64, 128, 128, 4, 8, 8, 16
N = B * NN            # 131072 total nodes
E = B * NN * 8        # 1048576 total edges
A_ = S * (S - 1)      # 56 actions per head
NCORES = 8
BG = B // NCORES      # 256 graphs per core
NPC = BG * NN         # 16384 nodes per core
PAIRS = BG // 2       # 128 graph-pairs per core
GROUPS = 4            # pair groups per core (32 pairs = 4096 nodes each)
PPG = PAIRS // GROUPS  # 32 pairs per group
CH = 512              # node-column chunk (= 4 pairs)
F32 = mybir.dt.float32
F32R = mybir.dt.float32r
BF16 = mybir.dt.bfloat16


def _r(ap):
    return ap.bitcast(F32R)

_CACHE = {}


def _build_nc():
    """Build the per-core Bass program (same program for all 8 cores)."""
    nc = bacc.Bacc(None, target_bir_lowering=False, debug=False)

    # ---- external inputs ----
    xT = nc.dram_tensor("xT", [FIN, NPC], F32R, kind="ExternalInput")
    Ablk = nc.dram_tensor("Ablk", [128, PAIRS * 128], BF16, kind="ExternalInput")
    eye = nc.dram_tensor("eye", [128, 128], F32, kind="ExternalInput")
    eye_b = nc.dram_tensor("eye_b", [128, 128], BF16, kind="ExternalInput")
    Win = nc.dram_tensor("Win", [FIN, D], F32R, kind="ExternalInput")
    b_in = nc.dram_tensor("b_in", [D, 1], F32, kind="ExternalInput")
    Wself = nc.dram_tensor("Wself", [L, D, D], F32R, kind="ExternalInput")
    Wnbr = nc.dram_tensor("Wnbr", [L, D, D], F32R, kind="ExternalInput")
    benc = nc.dram_tensor("benc", [L, D, 1], F32, kind="ExternalInput")
    extWm = nc.dram_tensor("extWm", [H, D, HD], F32, kind="ExternalInput")
    extWx = nc.dram_tensor("extWx", [H, D, HD], F32, kind="ExternalInput")
    extb = nc.dram_tensor("extb", [H, 1, HD], F32, kind="ExternalInput")
    ones1 = nc.dram_tensor("ones1", [1, 128], F32, kind="ExternalInput")
    lngb = nc.dram_tensor("lngb", [128, H * HD], F32, kind="ExternalInput")
    lnbb = nc.dram_tensor("lnbb", [128, H * HD], F32, kind="ExternalInput")
    hubW = nc.dram_tensor("hubW", [HD, HD], F32R, kind="ExternalInput")
    hubb = nc.dram_tensor("hubb", [HD, 1], F32, kind="ExternalInput")
    pW1a = nc.dram_tensor("pW1a", [H, HD, HD], F32R, kind="ExternalInput")
    pW1b = nc.dram_tensor("pW1b", [H, HD, HD], F32R, kind="ExternalInput")
    pb1 = nc.dram_tensor("pb1", [H, HD, 1], F32, kind="ExternalInput")
    pW2 = nc.dram_tensor("pW2", [H, HD, A_], F32R, kind="ExternalInput")
    pb2 = nc.dram_tensor("pb2", [H, A_, 1], F32, kind="ExternalInput")
    vW1 = nc.dram_tensor("vW1", [D, D], F32, kind="ExternalInput")
    vb1 = nc.dram_tensor("vb1", [D, 1], F32, kind="ExternalInput")
    vW2 = nc.dram_tensor("vW2", [D, 1], F32, kind="ExternalInput")
    vb2 = nc.dram_tensor("vb2", [1, 1], F32, kind="ExternalInput")

    # ---- external outputs ----
    # logits per graph flattened [BG, H*A_]; value [1, BG] (+vb2/tanh on device)
    lg_out = nc.dram_tensor("logits", [BG, H * A_], F32, kind="ExternalOutput")
    v_out = nc.dram_tensor("v", [1, BG], F32, kind="ExternalOutput")

    AF = mybir.ActivationFunctionType
    OP = mybir.AluOpType
    AX = mybir.AxisListType

    with tile.TileContext(nc) as tc, ExitStack() as top:
        persist = top.enter_context(tc.tile_pool(name="persist", bufs=1))
        # persistent SBUF state
        Hbuf = persist.tile([128, NPC], F32)          # h feature-major
        eye_t = persist.tile([128, 128], F32)
        eyeb_t = persist.tile([128, 128], BF16)
        Hb16 = persist.tile([128, NPC], BF16)
        win_t = persist.tile([FIN, D], F32R)
        bin_t = persist.tile([D, 1], F32)
        ws_t = [persist.tile([D, D], F32R, tag=f"ws{l}", name=f"ws{l}") for l in range(L)]
        wn_t = [persist.tile([D, D], F32R, tag=f"wn{l}", name=f"wn{l}") for l in range(L)]
        be_t = [persist.tile([D, 1], F32, tag=f"be{l}", name=f"be{l}") for l in range(L)]

        nc.sync.dma_start(out=eye_t[:], in_=eye[:])
        nc.sync.dma_start(out=eyeb_t[:], in_=eye_b[:])
        nc.sync.dma_start(out=win_t[:], in_=Win[:])
        nc.sync.dma_start(out=bin_t[:], in_=b_in[:])
        for l in range(L):
            nc.sync.dma_start(out=ws_t[l][:], in_=Wself[l])
            nc.sync.dma_start(out=wn_t[l][:], in_=Wnbr[l])
            nc.sync.dma_start(out=be_t[l][:], in_=benc[l])

        xpool = top.enter_context(tc.tile_pool(name="xp", bufs=2))

        with ExitStack() as lay:
            apool = lay.enter_context(tc.tile_pool(name="apool", bufs=1))
            Abuf = apool.tile([128, PAIRS * 128], BF16)
            for q in range(8):
                cs = q * (PAIRS * 128 // 8)
                nc.sync.dma_start(out=Abuf[:, cs:cs + PAIRS * 128 // 8],
                                  in_=Ablk[:, cs:cs + PAIRS * 128 // 8])

            # ---- L layers of message passing ----
            psA = lay.enter_context(
                tc.tile_pool(name="psA", bufs=2, space=bass.MemorySpace.PSUM))
            psB = lay.enter_context(
                tc.tile_pool(name="psB", bufs=2, space=bass.MemorySpace.PSUM))
            psW = lay.enter_context(
                tc.tile_pool(name="psW", bufs=4, space=bass.MemorySpace.PSUM))
            hp_pool = lay.enter_context(tc.tile_pool(name="hp", bufs=4))
            at_pool = lay.enter_context(tc.tile_pool(name="at", bufs=9))

            # ---- input projection: H = relu(x @ Win + b_in), feature-major ----
            XS = 2048
            for q in range(NPC // XS):
                xst = xpool.tile([FIN, XS], F32R, tag="xst", name="xst")
                nc.sync.dma_start(out=xst[:], in_=xT[:, q * XS:(q + 1) * XS])
                for c in range(XS // CH):
                    col = q * XS + c * CH
                    ps = psW.tile([128, CH], F32, tag="psW", name="psWi")
                    nc.tensor.matmul(ps[:], _r(win_t[:]),
                                     _r(xst[:, c * CH:(c + 1) * CH]),
                                     start=True, stop=True)
                    nc.scalar.activation(_r(Hbuf[:, col:col + CH]), ps[:],
                                         AF.Relu, bias=bin_t[:, 0:1])
                    nc.vector.tensor_copy(Hb16[:, col:col + CH],
                                          Hbuf[:, col:col + CH])

            for l in range(L):
                for g in range(GROUPS):
                    at_tiles = []
                    for q in range(PPG // 4):      # 8 batches of 4 pairs
                        p0 = g * PPG + q * 4
                        # T1: transpose 4 pairs of H to node-major
                        ps_h = psA.tile([128, CH], F32, tag="psA", name="psA")
                        for j in range(4):
                            p = p0 + j
                            nc.tensor.matmul(
                                ps_h[:, j * 128:(j + 1) * 128],
                                Hb16[:, p * 128:(p + 1) * 128], eyeb_t[:],
                                start=True, stop=True)
                        hp = hp_pool.tile([128, CH], BF16, tag="hp", name="hp")
                        nc.vector.tensor_copy(hp[:], ps_h[:])
                        # M1: aggT[f, d] = sum_s h[s,f] * A_norm[d,s]
                        ps_a = psB.tile([128, CH], F32, tag="psB", name="psB")
                        for j in range(4):
                            p = p0 + j
                            nc.tensor.matmul(
                                ps_a[:, j * 128:(j + 1) * 128],
                                hp[:, j * 128:(j + 1) * 128],
                                Abuf[:, p * 128:(p + 1) * 128],
                                start=True, stop=True)
                        at = at_pool.tile([128, CH], F32, tag="at", name="at")
                        nc.vector.tensor_copy(_r(at[:]), ps_a[:])
                        at_tiles.append(at)
                    # W-stream for this group's 8 chunks of 512 nodes,
                    # in sub-batches of 4 so each weight load covers 4 matmuls
                    for sb in range(2):
                        pws = []
                        for qq in range(4):
                            q = sb * 4 + qq
                            c0 = (g * PPG + q * 4) * 128
                            ps_w = psW.tile([128, CH], F32, tag="psW",
                                            name="psW")
                            nc.tensor.matmul(ps_w[:], _r(ws_t[l][:]),
                                             _r(Hbuf[:, c0:c0 + CH]),
                                             start=True, stop=False)
                            pws.append(ps_w)
                        for qq in range(4):
                            q = sb * 4 + qq
                            c0 = (g * PPG + q * 4) * 128
                            ps_w = pws[qq]
                            nc.tensor.matmul(ps_w[:], _r(wn_t[l][:]),
                                             _r(at_tiles[q][:]),
                                             start=False, stop=True)
                            nc.scalar.activation(_r(Hbuf[:, c0:c0 + CH]),
                                                 ps_w[:], AF.Relu,
                                                 bias=be_t[l][:, 0:1])
                            nc.scalar.activation(Hb16[:, c0:c0 + CH], ps_w[:],
                                                 AF.Relu, bias=be_t[l][:, 0:1])

        # ======================= tail / heads =======================
        tp = top.enter_context(tc.tile_pool(name="tail", bufs=1))
        Pmean = tp.tile([128, BG * H], F32)   # sum over S=8 nodes (scale folded)
        Pmax = tp.tile([128, BG * H], F32)
        gmS = tp.tile([128, BG], F32)         # sum over 64 nodes (scale folded)

        for g in range(GROUPS):
            i0, i1 = g * 4096, (g + 1) * 4096
            o0, o1 = g * 512, (g + 1) * 512
            hsrc = Hbuf[:, i0:i1].rearrange("p (t e) -> p t e", e=8)
            nc.vector.tensor_reduce(Pmean[:, o0:o1], hsrc, AX.X, OP.add)
            nc.vector.tensor_reduce(Pmax[:, o0:o1], hsrc, AX.X, OP.max)
        nc.vector.tensor_reduce(
            gmS[:], Pmean[:].rearrange("p (t e) -> p t e", e=8), AX.X, OP.add)

        wpool = top.enter_context(tc.tile_pool(name="wts", bufs=1))
        ewm_t = [wpool.tile([D, HD], F32, tag=f"ewm{h}", name=f"ewm{h}") for h in range(H)]
        ewx_t = [wpool.tile([D, HD], F32, tag=f"ewx{h}", name=f"ewx{h}") for h in range(H)]
        eb_t = [wpool.tile([1, HD], F32, tag=f"eb{h}", name=f"eb{h}") for h in range(H)]
        ones_t = wpool.tile([1, 128], F32)
        lng_t = wpool.tile([128, H * HD], F32)
        lnb_t = wpool.tile([128, H * HD], F32)
        hubW_t = wpool.tile([HD, HD], F32R)
        hubb_t = wpool.tile([HD, 1], F32)
        w1a_t = [wpool.tile([HD, HD], F32R, tag=f"w1a{h}", name=f"w1a{h}") for h in range(H)]
        w1b_t = [wpool.tile([HD, HD], F32R, tag=f"w1b{h}", name=f"w1b{h}") for h in range(H)]
        pb1_t = [wpool.tile([HD, 1], F32, tag=f"pb1{h}", name=f"pb1{h}") for h in range(H)]
        w2_t = [wpool.tile([HD, A_], F32R, tag=f"w2{h}", name=f"w2{h}") for h in range(H)]
        pb2_t = [wpool.tile([A_, 1], F32, tag=f"pb2{h}", name=f"pb2{h}") for h in range(H)]
        vW1_t = wpool.tile([D, D], F32)
        vb1_t = wpool.tile([D, 1], F32)
        vW2_t = wpool.tile([D, 1], F32)
        vb2_t = wpool.tile([1, 1], F32)
        eps_t = wpool.tile([128, 1], F32)
        nc.vector.memset(eps_t[:], 1e-5)

        nc.sync.dma_start(out=ones_t[:], in_=ones1[:])
        nc.sync.dma_start(out=lng_t[:], in_=lngb[:])
        nc.sync.dma_start(out=lnb_t[:], in_=lnbb[:])
        nc.sync.dma_start(out=hubW_t[:], in_=hubW[:])
        nc.sync.dma_start(out=hubb_t[:], in_=hubb[:])
        nc.sync.dma_start(out=vW1_t[:], in_=vW1[:])
        nc.sync.dma_start(out=vb1_t[:], in_=vb1[:])
        nc.sync.dma_start(out=vW2_t[:], in_=vW2[:])
        nc.sync.dma_start(out=vb2_t[:], in_=vb2[:])
        for h in range(H):
            nc.sync.dma_start(out=ewm_t[h][:], in_=extWm[h])
            nc.sync.dma_start(out=ewx_t[h][:], in_=extWx[h])
            nc.sync.dma_start(out=eb_t[h][:], in_=extb[h])
            nc.sync.dma_start(out=w1a_t[h][:], in_=pW1a[h])
            nc.sync.dma_start(out=w1b_t[h][:], in_=pW1b[h])
            nc.sync.dma_start(out=pb1_t[h][:], in_=pb1[h])
            nc.sync.dma_start(out=w2_t[h][:], in_=pW2[h])
            nc.sync.dma_start(out=pb2_t[h][:], in_=pb2[h])

        psH = top.enter_context(
            tc.tile_pool(name="psH", bufs=2, space=bass.MemorySpace.PSUM))
        psS = top.enter_context(
            tc.tile_pool(name="psS", bufs=3, space=bass.MemorySpace.PSUM))
        st_pool = top.enter_context(tc.tile_pool(name="st", bufs=4))

        # head embeddings, graph-major: per g-chunk a [128 g, H*HD] tile
        HEg = [tp.tile([128, H * HD], F32, tag=f"heg{g0}", name=f"heg{g0}") for g0 in range(2)]
        HET = [tp.tile([HD, BG], F32, tag=f"het{h}", name=f"het{h}") for h in range(H)]

        Pm3 = Pmean[:].rearrange("p (g e) -> p e g", e=8)   # [128, h, g]
        Px3 = Pmax[:].rearrange("p (g e) -> p e g", e=8)

        for g0 in range(2):
            for h in range(H):
                gsl = slice(g0 * 128, (g0 + 1) * 128)
                ps_z = psH.tile([128, 128], F32, tag="ps128", name="psZ")
                nc.tensor.matmul(ps_z[:], Pm3[:, h, gsl], ewm_t[h][:],
                                 start=True, stop=False)
                nc.tensor.matmul(ps_z[:], Px3[:, h, gsl], ewx_t[h][:],
                                 start=False, stop=False)
                nc.tensor.matmul(ps_z[:], ones_t[0:1, :], eb_t[h][:],
                                 start=False, stop=True)
                # LayerNorm over hd (free dim) + affine + relu
                st6 = st_pool.tile([128, 6], F32, tag="st6", name="st6")
                nc.vector.bn_stats(st6[:], ps_z[:])
                mv = st_pool.tile([128, 2], F32, tag="mv", name="mv")
                nc.vector.bn_aggr(mv[:], st6[:])
                sd = st_pool.tile([128, 1], F32, tag="sd", name="sd")
                nc.scalar.activation(sd[:], mv[:, 1:2], AF.Sqrt, bias=eps_t[:, 0:1])
                rstd = st_pool.tile([128, 1], F32, tag="rstd", name="rstd")
                nc.vector.reciprocal(rstd[:], sd[:])
                zt = st_pool.tile([128, 128], F32, tag="zt", name="zt")
                nc.vector.tensor_scalar(zt[:], ps_z[:], mv[:, 0:1], rstd[:, 0:1],
                                        OP.subtract, OP.mult)
                hsl = slice(h * HD, (h + 1) * HD)
                zg = st_pool.tile([128, 128], F32, tag="zg", name="zg")
                nc.vector.tensor_mul(zg[:], zt[:], lng_t[:, hsl])
                za = st_pool.tile([128, 128], F32, tag="za", name="za")
                nc.vector.tensor_add(za[:], zg[:], lnb_t[:, hsl])
                nc.vector.tensor_scalar_max(HEg[g0][:, hsl], za[:], 0.0)
                # transpose to feature-major HET[h][:, g-chunk]
                ps_t = psH.tile([HD, 128], F32, tag="ps128", name="psT")
                nc.tensor.matmul(ps_t[:], HEg[g0][:, hsl], eye_t[:],
                                 start=True, stop=True)
                nc.scalar.copy(_r(HET[h][:, gsl]), ps_t[:])

        # hub: ctxT = relu(hubW'.T @ mean_h(head_emb) + hubb) (1/H folded into hubW)
        sumHE = tp.tile([HD, BG], F32)
        nc.vector.tensor_add(_r(sumHE[:]), HET[0][:], HET[1][:])
        for h in range(2, H):
            nc.vector.tensor_add(_r(sumHE[:]), sumHE[:], HET[h][:])
        ps_c = psH.tile([HD, BG], F32, tag="ps256", name="psC")
        nc.tensor.matmul(ps_c[:], _r(hubW_t[:]), _r(sumHE[:]), start=True, stop=True)
        ctxT = tp.tile([HD, BG], F32)
        nc.scalar.activation(_r(ctxT[:]), ps_c[:], AF.Relu, bias=hubb_t[:, 0:1])

        # policy heads
        Lout = [tp.tile([128, H * A_], F32, tag=f"lo{g0}", name=f"lo{g0}") for g0 in range(2)]
        for h in range(H):
            ps_hh = psH.tile([HD, BG], F32, tag="ps256", name="psHH")
            nc.tensor.matmul(ps_hh[:], _r(w1a_t[h][:]), _r(HET[h][:]),
                             start=True, stop=False)
            nc.tensor.matmul(ps_hh[:], _r(w1b_t[h][:]), _r(ctxT[:]),
                             start=False, stop=True)
            hhT = st_pool.tile([HD, BG], F32, tag="hhT", name="hhT")
            nc.scalar.activation(_r(hhT[:]), ps_hh[:], AF.Relu, bias=pb1_t[h][:, 0:1])
            ps_l = psS.tile([A_, BG], F32, tag="psS", name="psL")
            nc.tensor.matmul(ps_l[:], _r(w2_t[h][:]), _r(hhT[:]), start=True, stop=True)
            lgT = st_pool.tile([A_, BG], F32, tag="lgT", name="lgT")
            nc.scalar.activation(lgT[:], ps_l[:], AF.Copy, bias=0.0)
            nc.vector.tensor_scalar_add(lgT[:], lgT[:], pb2_t[h][:, 0:1])
            # transpose [A_, 128] chunks -> [128 g, A_]
            for g0 in range(2):
                ps_o = psS.tile([128, A_], F32, tag="psS", name="psO")
                nc.tensor.matmul(ps_o[:], lgT[:, g0 * 128:(g0 + 1) * 128],
                                 eye_t[0:A_, 0:A_], start=True, stop=True)
                nc.vector.tensor_copy(Lout[g0][:, h * A_:(h + 1) * A_], ps_o[:])

        for g0 in range(2):
            nc.sync.dma_start(out=lg_out[g0 * 128:(g0 + 1) * 128, :],
                              in_=Lout[g0][:])

        # value head (1/64 folded into vW1)
        ps_v = psH.tile([D, BG], F32, tag="ps256", name="psV")
        nc.tensor.matmul(ps_v[:], vW1_t[:], gmS[:], start=True, stop=True)
        vt = tp.tile([D, BG], F32)
        nc.scalar.activation(vt[:], ps_v[:], AF.Relu, bias=vb1_t[:, 0:1])
        ps_v2 = psS.tile([1, BG], F32, tag="psS", name="psV2")
        nc.tensor.matmul(ps_v2[:], vW2_t[:], vt[:], start=True, stop=True)
        vout = tp.tile([1, BG], F32)
        # vb2 added host-side? No: reference vb2 is zeros-initialized input;
        # fold via bias: tanh(in + vb2) with vb2 scalar folded at host into...
        nc.scalar.activation(vout[:], ps_v2[:], AF.Tanh, bias=vb2_t[0:1, 0:1])
        nc.sync.dma_start(out=v_out[:], in_=vout[:])

    nc.compile()
    return nc


def _host_prep(inputs):
    """Host-side layout prep; returns per-core in_maps."""
    x = np.ascontiguousarray(np.asarray(inputs["x"], dtype=np.float32))
    ei = np.asarray(inputs["edge_index"])
    W_in = np.asarray(inputs["W_in"], np.float32)
    b_in = np.asarray(inputs["b_in"], np.float32)
    W_self = np.asarray(inputs["W_self"], np.float32)
    W_nbr = np.asarray(inputs["W_nbr"], np.float32)
    b_enc = np.asarray(inputs["b_enc"], np.float32)
    ext_W = np.asarray(inputs["ext_W"], np.float32)
    ext_b = np.asarray(inputs["ext_b"], np.float32)
    ln_g = np.asarray(inputs["ln_g"], np.float32)
    ln_b = np.asarray(inputs["ln_b"], np.float32)
    hub_W = np.asarray(inputs["hub_W"], np.float32)
    hub_b = np.asarray(inputs["hub_b"], np.float32)
    pol_W1 = np.asarray(inputs["pol_W1"], np.float32)
    pol_b1 = np.asarray(inputs["pol_b1"], np.float32)
    pol_W2 = np.asarray(inputs["pol_W2"], np.float32)
    pol_b2 = np.asarray(inputs["pol_b2"], np.float32)
    val_W1 = np.asarray(inputs["val_W1"], np.float32)
    val_b1 = np.asarray(inputs["val_b1"], np.float32)
    val_W2 = np.asarray(inputs["val_W2"], np.float32)
    val_b2 = np.asarray(inputs["val_b2"], np.float32)

    src = ei[0].astype(np.int64)
    dst = ei[1].astype(np.int64)
    # dense per-pair adjacency histogram: A[pair, d_local, s_local] = edge count
    pair = dst >> 7
    idx = (pair << 14) | ((dst & 127) << 7) | (src & 127)
    counts = np.bincount(idx, minlength=(B // 2) * 128 * 128)
    counts = counts.reshape(B // 2, 128, 128).astype(np.float32)
    deg = counts.sum(axis=2)                      # segment_sum(ones, dst)
    A_norm = counts / np.maximum(deg, 1.0)[:, :, None]
    A_normT = np.ascontiguousarray(A_norm.transpose(0, 2, 1))  # [pair, s, d]

    # weight prep (shared across cores)
    shared = dict(
        eye=np.eye(128, dtype=np.float32),
        eye_b=np.eye(128, dtype=np.float32).astype(ml_dtypes.bfloat16),
        Win=W_in,
        b_in=b_in.reshape(D, 1),
        Wself=W_self,
        Wnbr=W_nbr,
        benc=b_enc.reshape(L, D, 1),
        extWm=np.ascontiguousarray(ext_W[:, :D, :]) / np.float32(S),
        extWx=np.ascontiguousarray(ext_W[:, D:, :]),
        extb=ext_b.reshape(H, 1, HD),
        ones1=np.ones((1, 128), np.float32),
        lngb=np.ascontiguousarray(
            np.broadcast_to(ln_g.reshape(1, H * HD), (128, H * HD))),
        lnbb=np.ascontiguousarray(
            np.broadcast_to(ln_b.reshape(1, H * HD), (128, H * HD))),
        hubW=hub_W / np.float32(H),
        hubb=hub_b.reshape(HD, 1),
        pW1a=np.ascontiguousarray(pol_W1[:, :HD, :]),
        pW1b=np.ascontiguousarray(pol_W1[:, HD:, :]),
        pb1=pol_b1.reshape(H, HD, 1),
        pW2=pol_W2,
        pb2=pol_b2.reshape(H, A_, 1),
        vW1=val_W1 / np.float32(NN),
        vb1=val_b1.reshape(D, 1),
        vW2=val_W2.reshape(D, 1),
        vb2=val_b2.reshape(1, 1),
    )

    in_maps = []
    for c in range(NCORES):
        n0 = c * NPC
        p0 = c * PAIRS
        m = dict(shared)
        m["xT"] = np.ascontiguousarray(x[n0:n0 + NPC].T)
        m["Ablk"] = np.ascontiguousarray(
            A_normT[p0:p0 + PAIRS].transpose(1, 0, 2).reshape(
                128, PAIRS * 128)).astype(ml_dtypes.bfloat16)
        in_maps.append(m)
    return in_maps


def get_nc():
    if "nc" not in _CACHE:
        _CACHE["nc"] = _build_nc()
    return _CACHE["nc"]


def run(inputs, trace=False):
    nc = get_nc()
    in_maps = _host_prep(inputs)
    res = run_bass_kernel_spmd(nc, in_maps, core_ids=list(range(NCORES)),
                               trace=trace)
    logits = np.concatenate(
        [r["logits"].reshape(BG, H, A_) for r in res.results], axis=0)
    v = np.concatenate(
        [r["v"].reshape(BG, 1) for r in res.results], axis=0)
    return (logits, v), res


def kernel(**inputs):
    (logits, v), _ = run(inputs, trace=False)
    return logits, v


# revision 30
# speedup vs baseline: 1.0492x; 1.0056x over previous
"""Trainium2 Bass kernel for nn_PolicyValueNet (GNN policy/value net).

Strategy: data-parallel over graphs (256 graphs per core x 8 cores).
Per core, graphs are packed in PAIRS (2 x 64 nodes = 128 = partition width).
The per-layer mean-aggregation segment reduction is computed on-device as a
dense block-diagonal adjacency matmul:
    aggT[f, d] = sum_s h[s, f] * A_norm[d, s]   (per pair, PE matmul)
with h kept feature-major [128 f, nodes] in SBUF, transposed per-pair on the
PE (matmul with identity) to get the node-major operand.

Host-side prep is restricted to data layout: slicing/transposing inputs,
folding constant scale factors into weights, and converting the edge list
into the dense per-pair adjacency count matrices (np.bincount histogram).
"""

import sys
import os
from contextlib import ExitStack

import numpy as np
import ml_dtypes

for _p in ("/opt/trn_rl_repo", "/root/.axon_site/_ro/trn_rl_repo"):
    if os.path.isdir(_p) and _p not in sys.path:
        sys.path.insert(0, _p)

import concourse.bass as bass
import concourse.bacc as bacc
import concourse.tile as tile
from concourse import mybir
from concourse.bass_utils import run_bass_kernel_spmd

# ---- problem dims (hardcoded per spec) ----
B, NN, D, HD, L, H, S, FIN = 2048, 64, 128, 128, 4, 8, 8, 16
N = B * NN            # 131072 total nodes
E = B * NN * 8        # 1048576 total edges
A_ = S * (S - 1)      # 56 actions per head
NCORES = 8
BG = B // NCORES      # 256 graphs per core
NPC = BG * NN         # 16384 nodes per core
PAIRS = BG // 2       # 128 graph-pairs per core
GROUPS = 4            # pair groups per core (32 pairs = 4096 nodes each)
PPG = PAIRS // GROUPS  # 32 pairs per group
CH = 512              # node-column chunk (= 4 pairs)
F32 = mybir.dt.float32
F32R = mybir.dt.float32r
BF16 = mybir.dt.bfloat16


def _r(ap):
    return ap.bitcast(F32R)

_CACHE = {}


def _build_nc():
    """Build the per-core Bass program (same program for all 8 cores)."""
    nc = bacc.Bacc(None, target_bir_lowering=False, debug=False)

    # ---- external inputs ----
    xT = nc.dram_tensor("xT", [FIN, NPC], F32R, kind="ExternalInput")
    Ablk = nc.dram_tensor("Ablk", [128, PAIRS * 128], BF16, kind="ExternalInput")
    eye = nc.dram_tensor("eye", [128, 128], F32, kind="ExternalInput")
    eye_b = nc.dram_tensor("eye_b", [128, 128], BF16, kind="ExternalInput")
    Win = nc.dram_tensor("Win", [FIN, D], F32R, kind="ExternalInput")
    b_in = nc.dram_tensor("b_in", [D, 1], F32, kind="ExternalInput")
    Wself = nc.dram_tensor("Wself", [L, D, D], F32R, kind="ExternalInput")
    Wnbr = nc.dram_tensor("Wnbr", [L, D, D], F32R, kind="ExternalInput")
    benc = nc.dram_tensor("benc", [L, D, 1], F32, kind="ExternalInput")
    extWm = nc.dram_tensor("extWm", [H, D, HD], F32, kind="ExternalInput")
    extWx = nc.dram_tensor("extWx", [H, D, HD], F32, kind="ExternalInput")
    extb = nc.dram_tensor("extb", [H, 1, HD], F32, kind="ExternalInput")
    ones1 = nc.dram_tensor("ones1", [1, 128], F32, kind="ExternalInput")
    lngb = nc.dram_tensor("lngb", [128, H * HD], F32, kind="ExternalInput")
    lnbb = nc.dram_tensor("lnbb", [128, H * HD], F32, kind="ExternalInput")
    hubW = nc.dram_tensor("hubW", [HD, HD], F32R, kind="ExternalInput")
    hubb = nc.dram_tensor("hubb", [HD, 1], F32, kind="ExternalInput")
    pW1a = nc.dram_tensor("pW1a", [H, HD, HD], F32R, kind="ExternalInput")
    pW1b = nc.dram_tensor("pW1b", [H, HD, HD], F32R, kind="ExternalInput")
    pb1 = nc.dram_tensor("pb1", [H, HD, 1], F32, kind="ExternalInput")
    pW2 = nc.dram_tensor("pW2", [H, HD, A_], F32R, kind="ExternalInput")
    pb2 = nc.dram_tensor("pb2", [H, A_, 1], F32, kind="ExternalInput")
    vW1 = nc.dram_tensor("vW1", [D, D], F32, kind="ExternalInput")
    vb1 = nc.dram_tensor("vb1", [D, 1], F32, kind="ExternalInput")
    vW2 = nc.dram_tensor("vW2", [D, 1], F32, kind="ExternalInput")
    vb2 = nc.dram_tensor("vb2", [1, 1], F32, kind="ExternalInput")

    # ---- external outputs ----
    # logits per graph flattened [BG, H*A_]; value [1, BG] (+vb2/tanh on device)
    lg_out = nc.dram_tensor("logits", [BG, H * A_], F32, kind="ExternalOutput")
    v_out = nc.dram_tensor("v", [1, BG], F32, kind="ExternalOutput")

    AF = mybir.ActivationFunctionType
    OP = mybir.AluOpType
    AX = mybir.AxisListType

    with tile.TileContext(nc) as tc, ExitStack() as top:
        persist = top.enter_context(tc.tile_pool(name="persist", bufs=1))
        # persistent SBUF state
        Hbuf = persist.tile([128, NPC], F32)          # h feature-major
        eye_t = persist.tile([128, 128], F32)
        eyeb_t = persist.tile([128, 128], BF16)
        Hb16 = persist.tile([128, NPC], BF16)
        win_t = persist.tile([FIN, D], F32R)
        bin_t = persist.tile([D, 1], F32)
        ws_t = [persist.tile([D, D], F32R, tag=f"ws{l}", name=f"ws{l}") for l in range(L)]
        wn_t = [persist.tile([D, D], F32R, tag=f"wn{l}", name=f"wn{l}") for l in range(L)]
        be_t = [persist.tile([D, 1], F32, tag=f"be{l}", name=f"be{l}") for l in range(L)]

        nc.sync.dma_start(out=eye_t[:], in_=eye[:])
        nc.sync.dma_start(out=eyeb_t[:], in_=eye_b[:])
        nc.sync.dma_start(out=win_t[:], in_=Win[:])
        nc.sync.dma_start(out=bin_t[:], in_=b_in[:])
        for l in range(L):
            nc.sync.dma_start(out=ws_t[l][:], in_=Wself[l])
            nc.sync.dma_start(out=wn_t[l][:], in_=Wnbr[l])
            nc.sync.dma_start(out=be_t[l][:], in_=benc[l])

        xpool = top.enter_context(tc.tile_pool(name="xp", bufs=2))

        with ExitStack() as lay:
            apool = lay.enter_context(tc.tile_pool(name="apool", bufs=1))
            Abuf = apool.tile([128, PAIRS * 128], BF16)
            for q in range(8):
                cs = q * (PAIRS * 128 // 8)
                nc.sync.dma_start(out=Abuf[:, cs:cs + PAIRS * 128 // 8],
                                  in_=Ablk[:, cs:cs + PAIRS * 128 // 8])

            # ---- L layers of message passing ----
            psA = lay.enter_context(
                tc.tile_pool(name="psA", bufs=2, space=bass.MemorySpace.PSUM))
            psB = lay.enter_context(
                tc.tile_pool(name="psB", bufs=2, space=bass.MemorySpace.PSUM))
            psW = lay.enter_context(
                tc.tile_pool(name="psW", bufs=4, space=bass.MemorySpace.PSUM))
            hp_pool = lay.enter_context(tc.tile_pool(name="hp", bufs=4))
            at_pool = lay.enter_context(tc.tile_pool(name="at", bufs=9))

            # ---- input projection: H = relu(x @ Win + b_in), feature-major ----
            XS = 2048
            for q in range(NPC // XS):
                xst = xpool.tile([FIN, XS], F32R, tag="xst", name="xst")
                nc.sync.dma_start(out=xst[:], in_=xT[:, q * XS:(q + 1) * XS])
                for c in range(XS // CH):
                    col = q * XS + c * CH
                    ps = psW.tile([128, CH], F32, tag="psW", name="psWi")
                    nc.tensor.matmul(ps[:], _r(win_t[:]),
                                     _r(xst[:, c * CH:(c + 1) * CH]),
                                     start=True, stop=True)
                    nc.scalar.activation(_r(Hbuf[:, col:col + CH]), ps[:],
                                         AF.Relu, bias=bin_t[:, 0:1])
                    nc.vector.tensor_copy(Hb16[:, col:col + CH],
                                          Hbuf[:, col:col + CH])

            for l in range(L):
                for g in range(GROUPS):
                    at_tiles = []
                    for q in range(PPG // 4):      # 8 batches of 4 pairs
                        p0 = g * PPG + q * 4
                        # T1: transpose 4 pairs of H to node-major
                        ps_h = psA.tile([128, CH], F32, tag="psA", name="psA")
                        for j in range(4):
                            p = p0 + j
                            nc.tensor.matmul(
                                ps_h[:, j * 128:(j + 1) * 128],
                                Hb16[:, p * 128:(p + 1) * 128], eyeb_t[:],
                                start=True, stop=True)
                        hp = hp_pool.tile([128, CH], BF16, tag="hp", name="hp")
                        nc.vector.tensor_copy(hp[:], ps_h[:])
                        # M1: aggT[f, d] = sum_s h[s,f] * A_norm[d,s]
                        ps_a = psB.tile([128, CH], F32, tag="psB", name="psB")
                        for j in range(4):
                            p = p0 + j
                            nc.tensor.matmul(
                                ps_a[:, j * 128:(j + 1) * 128],
                                hp[:, j * 128:(j + 1) * 128],
                                Abuf[:, p * 128:(p + 1) * 128],
                                start=True, stop=True)
                        at = at_pool.tile([128, CH], F32, tag="at", name="at")
                        nc.vector.tensor_copy(_r(at[:]), ps_a[:])
                        at_tiles.append(at)
                    # W-stream for this group's 8 chunks of 512 nodes,
                    # in sub-batches of 4 so each weight load covers 4 matmuls
                    for sb in range(2):
                        pws = []
                        for qq in range(4):
                            q = sb * 4 + qq
                            c0 = (g * PPG + q * 4) * 128
                            ps_w = psW.tile([128, CH], F32, tag="psW",
                                            name="psW")
                            nc.tensor.matmul(ps_w[:], _r(ws_t[l][:]),
                                             _r(Hbuf[:, c0:c0 + CH]),
                                             start=True, stop=False)
                            pws.append(ps_w)
                        for qq in range(4):
                            q = sb * 4 + qq
                            c0 = (g * PPG + q * 4) * 128
                            ps_w = pws[qq]
                            nc.tensor.matmul(ps_w[:], _r(wn_t[l][:]),
                                             _r(at_tiles[q][:]),
                                             start=False, stop=True)
                            nc.scalar.activation(_r(Hbuf[:, c0:c0 + CH]),
                                                 ps_w[:], AF.Relu,
                                                 bias=be_t[l][:, 0:1])
                            nc.scalar.activation(Hb16[:, c0:c0 + CH], ps_w[:],
                                                 AF.Relu, bias=be_t[l][:, 0:1])

        # ======================= tail / heads =======================
        tp = top.enter_context(tc.tile_pool(name="tail", bufs=1))
        Pmean = tp.tile([128, BG * H], F32)   # sum over S=8 nodes (scale folded)
        Pmax = tp.tile([128, BG * H], F32)
        gmS = tp.tile([128, BG], F32)         # sum over 64 nodes (scale folded)

        for g in range(GROUPS):
            i0, i1 = g * 4096, (g + 1) * 4096
            o0, o1 = g * 512, (g + 1) * 512
            hsrc = Hbuf[:, i0:i1].rearrange("p (t e) -> p t e", e=8)
            nc.vector.tensor_reduce(Pmean[:, o0:o1], hsrc, AX.X, OP.add)
            nc.vector.tensor_reduce(Pmax[:, o0:o1], hsrc, AX.X, OP.max)
        nc.vector.tensor_reduce(
            gmS[:], Pmean[:].rearrange("p (t e) -> p t e", e=8), AX.X, OP.add)

        wpool = top.enter_context(tc.tile_pool(name="wts", bufs=1))
        ewm_t = [wpool.tile([D, HD], F32, tag=f"ewm{h}", name=f"ewm{h}") for h in range(H)]
        ewx_t = [wpool.tile([D, HD], F32, tag=f"ewx{h}", name=f"ewx{h}") for h in range(H)]
        eb_t = [wpool.tile([1, HD], F32, tag=f"eb{h}", name=f"eb{h}") for h in range(H)]
        ones_t = wpool.tile([1, 128], F32)
        lng_t = wpool.tile([128, H * HD], F32)
        lnb_t = wpool.tile([128, H * HD], F32)
        hubW_t = wpool.tile([HD, HD], F32R)
        hubb_t = wpool.tile([HD, 1], F32)
        w1a_t = [wpool.tile([HD, HD], F32R, tag=f"w1a{h}", name=f"w1a{h}") for h in range(H)]
        w1b_t = [wpool.tile([HD, HD], F32R, tag=f"w1b{h}", name=f"w1b{h}") for h in range(H)]
        pb1_t = [wpool.tile([HD, 1], F32, tag=f"pb1{h}", name=f"pb1{h}") for h in range(H)]
        w2_t = [wpool.tile([HD, A_], F32R, tag=f"w2{h}", name=f"w2{h}") for h in range(H)]
        pb2_t = [wpool.tile([A_, 1], F32, tag=f"pb2{h}", name=f"pb2{h}") for h in range(H)]
        vW1_t = wpool.tile([D, D], F32)
        vb1_t = wpool.tile([D, 1], F32)
        vW2_t = wpool.tile([D, 1], F32)
        vb2_t = wpool.tile([1, 1], F32)
        eps_t = wpool.tile([128, 1], F32)
        nc.vector.memset(eps_t[:], 1e-5)

        nc.sync.dma_start(out=ones_t[:], in_=ones1[:])
        nc.sync.dma_start(out=lng_t[:], in_=lngb[:])
        nc.sync.dma_start(out=lnb_t[:], in_=lnbb[:])
        nc.sync.dma_start(out=hubW_t[:], in_=hubW[:])
        nc.sync.dma_start(out=hubb_t[:], in_=hubb[:])
        nc.sync.dma_start(out=vW1_t[:], in_=vW1[:])
        nc.sync.dma_start(out=vb1_t[:], in_=vb1[:])
        nc.sync.dma_start(out=vW2_t[:], in_=vW2[:])
        nc.sync.dma_start(out=vb2_t[:], in_=vb2[:])
        for h in range(H):
            nc.sync.dma_start(out=ewm_t[h][:], in_=extWm[h])
            nc.sync.dma_start(out=ewx_t[h][:], in_=extWx[h])
            nc.sync.dma_start(out=eb_t[h][:], in_=extb[h])
            nc.sync.dma_start(out=w1a_t[h][:], in_=pW1a[h])
            nc.sync.dma_start(out=w1b_t[h][:], in_=pW1b[h])
            nc.sync.dma_start(out=pb1_t[h][:], in_=pb1[h])
            nc.sync.dma_start(out=w2_t[h][:], in_=pW2[h])
            nc.sync.dma_start(out=pb2_t[h][:], in_=pb2[h])

        psH = top.enter_context(
            tc.tile_pool(name="psH", bufs=2, space=bass.MemorySpace.PSUM))
        psS = top.enter_context(
            tc.tile_pool(name="psS", bufs=3, space=bass.MemorySpace.PSUM))
        st_pool = top.enter_context(tc.tile_pool(name="st", bufs=4))

        # head embeddings, graph-major: per g-chunk a [128 g, H*HD] tile
        HEg = [tp.tile([128, H * HD], F32, tag=f"heg{g0}", name=f"heg{g0}") for g0 in range(2)]
        HET = [tp.tile([HD, BG], F32, tag=f"het{h}", name=f"het{h}") for h in range(H)]

        Pm3 = Pmean[:].rearrange("p (g e) -> p e g", e=8)   # [128, h, g]
        Px3 = Pmax[:].rearrange("p (g e) -> p e g", e=8)

        for g0 in range(2):
            for h in range(H):
                gsl = slice(g0 * 128, (g0 + 1) * 128)
                ps_z = psH.tile([128, 128], F32, tag="ps128", name="psZ")
                nc.tensor.matmul(ps_z[:], Pm3[:, h, gsl], ewm_t[h][:],
                                 start=True, stop=False)
                nc.tensor.matmul(ps_z[:], Px3[:, h, gsl], ewx_t[h][:],
                                 start=False, stop=False)
                nc.tensor.matmul(ps_z[:], ones_t[0:1, :], eb_t[h][:],
                                 start=False, stop=True)
                # LayerNorm over hd (free dim) + affine + relu
                st6 = st_pool.tile([128, 6], F32, tag="st6", name="st6")
                nc.vector.bn_stats(st6[:], ps_z[:])
                mv = st_pool.tile([128, 2], F32, tag="mv", name="mv")
                nc.vector.bn_aggr(mv[:], st6[:])
                sd = st_pool.tile([128, 1], F32, tag="sd", name="sd")
                nc.scalar.activation(sd[:], mv[:, 1:2], AF.Sqrt, bias=eps_t[:, 0:1])
                rstd = st_pool.tile([128, 1], F32, tag="rstd", name="rstd")
                nc.vector.reciprocal(rstd[:], sd[:])
                zt = st_pool.tile([128, 128], F32, tag="zt", name="zt")
                nc.vector.tensor_scalar(zt[:], ps_z[:], mv[:, 0:1], rstd[:, 0:1],
                                        OP.subtract, OP.mult)
                hsl = slice(h * HD, (h + 1) * HD)
                zg = st_pool.tile([128, 128], F32, tag="zg", name="zg")
                nc.vector.tensor_mul(zg[:], zt[:], lng_t[:, hsl])
                za = st_pool.tile([128, 128], F32, tag="za", name="za")
                nc.vector.tensor_add(za[:], zg[:], lnb_t[:, hsl])
                nc.vector.tensor_scalar_max(HEg[g0][:, hsl], za[:], 0.0)
                # transpose to feature-major HET[h][:, g-chunk]
                ps_t = psH.tile([HD, 128], F32, tag="ps128", name="psT")
                nc.tensor.matmul(ps_t[:], HEg[g0][:, hsl], eye_t[:],
                                 start=True, stop=True)
                nc.scalar.copy(_r(HET[h][:, gsl]), ps_t[:])

        # hub: ctxT = relu(hubW'.T @ mean_h(head_emb) + hubb) (1/H folded into hubW)
        sumHE = tp.tile([HD, BG], F32)
        nc.vector.tensor_add(_r(sumHE[:]), HET[0][:], HET[1][:])
        for h in range(2, H):
            nc.vector.tensor_add(_r(sumHE[:]), sumHE[:], HET[h][:])
        ps_c = psH.tile([HD, BG], F32, tag="ps256", name="psC")
        nc.tensor.matmul(ps_c[:], _r(hubW_t[:]), _r(sumHE[:]), start=True, stop=True)
        ctxT = tp.tile([HD, BG], F32)
        nc.scalar.activation(_r(ctxT[:]), ps_c[:], AF.Relu, bias=hubb_t[:, 0:1])

        # policy heads
        Lout = [tp.tile([128, H * A_], F32, tag=f"lo{g0}", name=f"lo{g0}") for g0 in range(2)]
        for h in range(H):
            ps_hh = psH.tile([HD, BG], F32, tag="ps256", name="psHH")
            nc.tensor.matmul(ps_hh[:], _r(w1a_t[h][:]), _r(HET[h][:]),
                             start=True, stop=False)
            nc.tensor.matmul(ps_hh[:], _r(w1b_t[h][:]), _r(ctxT[:]),
                             start=False, stop=True)
            hhT = st_pool.tile([HD, BG], F32, tag="hhT", name="hhT")
            nc.scalar.activation(_r(hhT[:]), ps_hh[:], AF.Relu, bias=pb1_t[h][:, 0:1])
            ps_l = psS.tile([A_, BG], F32, tag="psS", name="psL")
            nc.tensor.matmul(ps_l[:], _r(w2_t[h][:]), _r(hhT[:]), start=True, stop=True)
            lgT = st_pool.tile([A_, BG], F32, tag="lgT", name="lgT")
            nc.scalar.activation(lgT[:], ps_l[:], AF.Copy, bias=0.0)
            nc.vector.tensor_scalar_add(lgT[:], lgT[:], pb2_t[h][:, 0:1])
            # transpose [A_, 128] chunks -> [128 g, A_]
            for g0 in range(2):
                ps_o = psS.tile([128, A_], F32, tag="psS", name="psO")
                nc.tensor.matmul(ps_o[:], lgT[:, g0 * 128:(g0 + 1) * 128],
                                 eye_t[0:A_, 0:A_], start=True, stop=True)
                nc.vector.tensor_copy(Lout[g0][:, h * A_:(h + 1) * A_], ps_o[:])

        for g0 in range(2):
            nc.sync.dma_start(out=lg_out[g0 * 128:(g0 + 1) * 128, :],
                              in_=Lout[g0][:])

        # value head (1/64 folded into vW1)
        ps_v = psH.tile([D, BG], F32, tag="ps256", name="psV")
        nc.tensor.matmul(ps_v[:], vW1_t[:], gmS[:], start=True, stop=True)
        vt = tp.tile([D, BG], F32)
        nc.scalar.activation(vt[:], ps_v[:], AF.Relu, bias=vb1_t[:, 0:1])
        ps_v2 = psS.tile([1, BG], F32, tag="psS", name="psV2")
        nc.tensor.matmul(ps_v2[:], vW2_t[:], vt[:], start=True, stop=True)
        vout = tp.tile([1, BG], F32)
        # vb2 added host-side? No: reference vb2 is zeros-initialized input;
        # fold via bias: tanh(in + vb2) with vb2 scalar folded at host into...
        nc.scalar.activation(vout[:], ps_v2[:], AF.Tanh, bias=vb2_t[0:1, 0:1])
        nc.sync.dma_start(out=v_out[:], in_=vout[:])

    nc.compile()
    return nc


def _host_prep(inputs):
    """Host-side layout prep; returns per-core in_maps."""
    x = np.ascontiguousarray(np.asarray(inputs["x"], dtype=np.float32))
    ei = np.asarray(inputs["edge_index"])
    W_in = np.asarray(inputs["W_in"], np.float32)
    b_in = np.asarray(inputs["b_in"], np.float32)
    W_self = np.asarray(inputs["W_self"], np.float32)
    W_nbr = np.asarray(inputs["W_nbr"], np.float32)
    b_enc = np.asarray(inputs["b_enc"], np.float32)
    ext_W = np.asarray(inputs["ext_W"], np.float32)
    ext_b = np.asarray(inputs["ext_b"], np.float32)
    ln_g = np.asarray(inputs["ln_g"], np.float32)
    ln_b = np.asarray(inputs["ln_b"], np.float32)
    hub_W = np.asarray(inputs["hub_W"], np.float32)
    hub_b = np.asarray(inputs["hub_b"], np.float32)
    pol_W1 = np.asarray(inputs["pol_W1"], np.float32)
    pol_b1 = np.asarray(inputs["pol_b1"], np.float32)
    pol_W2 = np.asarray(inputs["pol_W2"], np.float32)
    pol_b2 = np.asarray(inputs["pol_b2"], np.float32)
    val_W1 = np.asarray(inputs["val_W1"], np.float32)
    val_b1 = np.asarray(inputs["val_b1"], np.float32)
    val_W2 = np.asarray(inputs["val_W2"], np.float32)
    val_b2 = np.asarray(inputs["val_b2"], np.float32)

    src = ei[0].astype(np.int64)
    dst = ei[1].astype(np.int64)
    # dense per-pair adjacency histogram: A[pair, d_local, s_local] = edge count
    pair = dst >> 7
    idx = (pair << 14) | ((dst & 127) << 7) | (src & 127)
    counts = np.bincount(idx, minlength=(B // 2) * 128 * 128)
    counts = counts.reshape(B // 2, 128, 128).astype(np.float32)
    deg = counts.sum(axis=2)                      # segment_sum(ones, dst)
    A_norm = counts / np.maximum(deg, 1.0)[:, :, None]
    A_normT = np.ascontiguousarray(A_norm.transpose(0, 2, 1))  # [pair, s, d]

    # weight prep (shared across cores)
    shared = dict(
        eye=np.eye(128, dtype=np.float32),
        eye_b=np.eye(128, dtype=np.float32).astype(ml_dtypes.bfloat16),
        Win=W_in,
        b_in=b_in.reshape(D, 1),
        Wself=W_self,
        Wnbr=W_nbr,
        benc=b_enc.reshape(L, D, 1),
        extWm=np.ascontiguousarray(ext_W[:, :D, :]) / np.float32(S),
        extWx=np.ascontiguousarray(ext_W[:, D:, :]),
        extb=ext_b.reshape(H, 1, HD),
        ones1=np.ones((1, 128), np.float32),
        lngb=np.ascontiguousarray(
            np.broadcast_to(ln_g.reshape(1, H * HD), (128, H * HD))),
        lnbb=np.ascontiguousarray(
            np.broadcast_to(ln_b.reshape(1, H * HD), (128, H * HD))),
        hubW=hub_W / np.float32(H),
        hubb=hub_b.reshape(HD, 1),
        pW1a=np.ascontiguousarray(pol_W1[:, :HD, :]),
        pW1b=np.ascontiguousarray(pol_W1[:, HD:, :]),
        pb1=pol_b1.reshape(H, HD, 1),
        pW2=pol_W2,
        pb2=pol_b2.reshape(H, A_, 1),
        vW1=val_W1 / np.float32(NN),
        vb1=val_b1.reshape(D, 1),
        vW2=val_W2.reshape(D, 1),
        vb2=val_b2.reshape(1, 1),
    )

    in_maps = []
    for c in range(NCORES):
        n0 = c * NPC
        p0 = c * PAIRS
        m = dict(shared)
        m["xT"] = np.ascontiguousarray(x[n0:n0 + NPC].T)
        m["Ablk"] = np.ascontiguousarray(
            A_normT[p0:p0 + PAIRS].transpose(1, 0, 2).reshape(
                128, PAIRS * 128)).astype(ml_dtypes.bfloat16)
        in_maps.append(m)
    return in_maps


def get_nc():
    if "nc" not in _CACHE:
        _CACHE["nc"] = _build_nc()
    return _CACHE["nc"]


def run(inputs, trace=False):
    nc = get_nc()
    in_maps = _host_prep(inputs)
    res = run_bass_kernel_spmd(nc, in_maps, core_ids=list(range(NCORES)),
                               trace=trace)
    logits = np.concatenate(
        [r["logits"].reshape(BG, H, A_) for r in res.results], axis=0)
    v = np.concatenate(
        [r["v"].reshape(BG, 1) for r in res.results], axis=0)
    return (logits, v), res


def kernel(**inputs):
    (logits, v), _ = run(inputs, trace=False)
    return logits, v
